# revision 7
# baseline (speedup 1.0000x reference)
"""Trainium2 Bass kernel for the bipartite GNN message-passing encoder.

Math (see reference.py):
  A_r = (adj == r), r = 1..5
  An_r = diag(1/sqrt(Nu)) A_r diag(1/sqrt(Nv))   (exact factorization; the
         Csafe guard in the reference only matters where A==0, contributing 0)
  Hu = relu(sum_r An_r @ W_items_r^T)   [NU, M]
  Hv = relu(sum_r An_r^T @ W_users_r^T) [NI, M]
  U  = relu(Hu @ dense_W^T + relu(u_sideFeat @ u_W1^T + u_b1) @ u_W2^T)
  V  = relu(Hv @ dense_W^T + relu(v_sideFeat @ v_W1^T + v_b1) @ v_W2^T)

Sharding: 4 user-groups x 2 item-groups = 8 cores. Core (a, b) holds the
adjacency block adj[a*1000:(a+1)*1000, b*2000:(b+1)*2000] and computes the
partial Hu^T for its 1000 users (partial over items -> AllReduce over the
pair sharing `a`) and the partial Hv^T for its 2000 items (partial over
users -> AllReduce over the quad sharing `b`). Degrees (Nu/Nv) are computed
on-device from the blocks with two tiny AllReduces. The inner degree scale
is folded into the mask build (one dual-op DVE instruction per mask tile);
the outer degree scale is applied in pass 2. Pass 2 (tiny dense layers) is
computed redundantly inside each reduce group so the SPMD program has no
per-core constants.
"""

import sys

import numpy as np

if "/opt/trn_rl_repo" not in sys.path:
    sys.path.insert(0, "/opt/trn_rl_repo")

import concourse.bass as bass  # noqa: E402
import concourse.bacc as bacc  # noqa: E402
import concourse.mybir as mybir  # noqa: E402
import concourse.tile as tile  # noqa: E402
from concourse.masks import make_identity  # noqa: E402

FP = mybir.dt.float32
BF = mybir.dt.bfloat16
I32 = mybir.dt.int32

NU = NI = 4000
R = 5
M = 256
OUT = 75
SIDE = 64
FDIM = 128

GA, GB = 4, 2  # user groups x item groups
BU = NU // GA  # 1000 users per block
BI = NI // GB  # 2000 items per block
NCORES = GA * GB

AF = mybir.ActivationFunctionType
ALU = mybir.AluOpType

# AllReduce groups (core id = a*GB + b)
PAIR_GROUPS = [[a * GB, a * GB + 1] for a in range(GA)]  # share users (same a)
QUAD_GROUPS = [[b, GB + b, 2 * GB + b, 3 * GB + b] for b in range(GB)]  # same b


def _ptiles(n, p=128):
    return [(s, min(p, n - s)) for s in range(0, n, p)]


UPT = _ptiles(BU)  # 8 tiles over block users
IPT = _ptiles(BI)  # 16 tiles over block items


def build_program():
    from contextlib import ExitStack

    nc = bacc.Bacc("TRN2", target_bir_lowering=False, debug=False, num_devices=NCORES)

    # ---- I/O ----
    adj_blk = nc.dram_tensor("adj_blk", [BU, BI], I32, kind="ExternalInput")
    wi = nc.dram_tensor("wi", [R, M, BI], FP, kind="ExternalInput")
    wu = nc.dram_tensor("wu", [R, M, BU], FP, kind="ExternalInput")
    uf = nc.dram_tensor("uf", [BU, FDIM], FP, kind="ExternalInput")
    vf = nc.dram_tensor("vf", [BI, FDIM], FP, kind="ExternalInput")
    dw = nc.dram_tensor("dw", [OUT, M], FP, kind="ExternalInput")
    uw1 = nc.dram_tensor("uw1", [SIDE, FDIM], FP, kind="ExternalInput")
    ub1 = nc.dram_tensor("ub1", [SIDE, 1], FP, kind="ExternalInput")
    uw2 = nc.dram_tensor("uw2", [OUT, SIDE], FP, kind="ExternalInput")
    vw1 = nc.dram_tensor("vw1", [SIDE, FDIM], FP, kind="ExternalInput")
    vb1 = nc.dram_tensor("vb1", [SIDE, 1], FP, kind="ExternalInput")
    vw2 = nc.dram_tensor("vw2", [OUT, SIDE], FP, kind="ExternalInput")
    u_out = nc.dram_tensor("u_out", [BU, OUT], FP, kind="ExternalOutput")
    v_out = nc.dram_tensor("v_out", [BI, OUT], FP, kind="ExternalOutput")

    with tile.TileContext(nc) as tc, ExitStack() as ctx:
        res = ctx.enter_context(tc.tile_pool(name="res", bufs=1))
        adjp = ctx.enter_context(tc.tile_pool(name="adjp", bufs=1))
        scr = ctx.enter_context(tc.tile_pool(name="scr", bufs=2))
        dram = ctx.enter_context(tc.tile_pool(name="dram", bufs=1, space="DRAM"))
        ps_tr = ctx.enter_context(tc.tile_pool(name="ps_tr", bufs=2, space="PSUM"))
        ps_cs = tc.alloc_tile_pool(name="ps_cs", bufs=2, space="PSUM")
        ps_mm = tc.alloc_tile_pool(name="ps_mm", bufs=4, space="PSUM")

        # ---- constants ----
        ident = res.tile([128, 128], BF, tag="ident")
        make_identity(nc, ident[:])
        ones = res.tile([128, 1], BF, tag="ones")
        nc.gpsimd.memset(ones[:], 1.0)

        # =========== Phase 1: adj load/convert, degrees ===========
        adjb = []  # bf16 [pu, BI] resident, one per user ptile
        rd_t = []  # row degree [pu, 1] f32 per user ptile
        cd_acc = res.tile([1, BI], FP, tag="cd_acc")
        nc.gpsimd.memset(cd_acc[:], 0.0)
        for t, (s, pu) in enumerate(UPT):
            ab = res.tile([128, BI], BF, tag=f"adjb{t}")
            adjb.append(ab)
            rd = res.tile([128, 1], FP, tag=f"rd{t}")
            rd_t.append(rd)
            rdc = []
            for c in (0, 1000):
                ai = scr.tile([128, 1000], I32, tag="ai")
                nc.sync.dma_start(out=ai[:pu, :], in_=adj_blk[s : s + pu, c : c + 1000])
                nc.gpsimd.tensor_copy(out=ab[:pu, c : c + 1000], in_=ai[:pu, :])
                # nonzero mask (= min(adj,1)) + row-degree partial via accumulate
                nz = scr.tile([128, 1000], BF, tag="nz")
                rc = scr.tile([128, 1], FP, tag="rdc")
                nc.vector.tensor_scalar(
                    out=nz[:pu, :], in0=ab[:pu, c : c + 1000], scalar1=1.0,
                    scalar2=None, op0=ALU.min, op1=ALU.add, accum_out=rc[:pu, :],
                )
                rdc.append(rc)
                # column-degree partials via ones-matmul
                for h in (0, 500):
                    cs = ps_cs.tile([1, 500], FP, tag="cs")
                    nc.tensor.matmul(
                        cs[:1, :], lhsT=ones[:pu, :1], rhs=nz[:pu, h : h + 500],
                        start=True, stop=True,
                    )
                    nc.vector.tensor_tensor(
                        out=cd_acc[:1, c + h : c + h + 500],
                        in0=cd_acc[:1, c + h : c + h + 500],
                        in1=cs[:1, :], op=ALU.add,
                    )
            nc.vector.tensor_tensor(
                out=rd[:pu, :], in0=rdc[0][:pu, :], in1=rdc[1][:pu, :], op=ALU.add
            )

        # degree AllReduces
        dram_rd = dram.tile([BU, 1], FP, tag="dram_rd")
        dram_cd = dram.tile([1, BI], FP, tag="dram_cd")
        dram_rd_red = dram.tile([BU, 1], FP, tag="dram_rd_red")
        dram_cd_red = dram.tile([1, BI], FP, tag="dram_cd_red")
        for t, (s, pu) in enumerate(UPT):
            nc.sync.dma_start(out=dram_rd[s : s + pu, :], in_=rd_t[t][:pu, :])
        nc.sync.dma_start(out=dram_cd[:, :], in_=cd_acc[:1, :])
        nc.gpsimd.collective_compute(
            "AllReduce", ALU.add, replica_groups=PAIR_GROUPS,
            ins=[dram_rd.opt()], outs=[dram_rd_red.opt()],
        )
        nc.gpsimd.collective_compute(
            "AllReduce", ALU.add, replica_groups=QUAD_GROUPS,
            ins=[dram_cd.opt()], outs=[dram_cd_red.opt()],
        )

        # a = rsqrt(max(Nu,1)) per user ptile; b = rsqrt(max(Nv,1)) per item ptile
        def rsqrt_tiles(src_rows, tiles, nm):
            # src_rows: callable t -> AP of shape [p, 1] in DRAM
            out = []
            for t, (s, p) in enumerate(tiles):
                raw = scr.tile([128, 1], FP, tag="fraw")
                nc.sync.dma_start(out=raw[:p, :], in_=src_rows(s, p))
                m1 = scr.tile([128, 1], FP, tag="fm1")
                nc.vector.tensor_scalar(
                    out=m1[:p, :], in0=raw[:p, :], scalar1=1.0, scalar2=None,
                    op0=ALU.max,
                )
                sq = scr.tile([128, 1], FP, tag="fsq")
                nc.scalar.sqrt(out=sq[:p, :], in_=m1[:p, :])
                fac = res.tile([128, 1], FP, tag=f"{nm}fac{t}")
                nc.vector.reciprocal(out=fac[:p, :], in_=sq[:p, :])
                out.append(fac)
            return out

        a_fac = rsqrt_tiles(lambda s, p: dram_rd_red[s : s + p, :], UPT, "a")
        b_fac = rsqrt_tiles(lambda s, p: dram_cd_red[:, s : s + p], IPT, "b")

        # =========== Phase 3: adj transpose (bf16) ===========
        adjT = []  # [pi, BU] bf16, one per item ptile
        for t, (s, pi) in enumerate(IPT):
            at = adjp.tile([128, BU], BF, tag=f"adjT{t}")
            adjT.append(at)
            for j2 in range(0, len(UPT), 2):
                pt_ps = ps_tr.tile([128, 256], BF, tag="trp")
                w = 0
                for j in (j2, j2 + 1):
                    us, pu = UPT[j]
                    nc.tensor.transpose(
                        pt_ps[:pi, w : w + pu],
                        adjb[j][:pu, s : s + pi],
                        ident[:pu, :pu],
                    )
                    w += pu
                nc.scalar.copy(
                    out=at[:pi, UPT[j2][0] : UPT[j2][0] + w], in_=pt_ps[:pi, :w]
                )

        # helper: JIT W transpose prep.  dest[:p, :M] <- bf16(w_dram[r, :, s:s+p]).T
        def prep_wT(w_dram, r, s, p, dest):
            pt_ps = ps_tr.tile([128, 256], BF, tag="trp")
            for mh in range(2):
                wf = scr.tile([128, 128], FP, tag="wf", bufs=3)
                nc.sync.dma_start(
                    out=wf[:, :p], in_=w_dram[r, mh * 128 : (mh + 1) * 128, s : s + p]
                )
                wb = scr.tile([128, 128], BF, tag="wb", bufs=3)
                nc.scalar.copy(out=wb[:, :p], in_=wf[:, :p])
                nc.tensor.transpose(
                    pt_ps[:p, mh * 128 : mh * 128 + 128], wb[:, :p], ident[:, :]
                )
            nc.scalar.copy(out=dest[:p, :], in_=pt_ps[:p, :M])

        # DRAM buffers for pass-1 partials
        dram_huT = dram.tile([M, BU], FP, tag="dram_huT")
        dram_hvT = dram.tile([M, BI], FP, tag="dram_hvT")
        dram_huT_red = dram.tile([M, BU], FP, tag="dram_huT_red")
        dram_hvT_red = dram.tile([M, BI], FP, tag="dram_hvT_red")

        # =========== ITEM-side pass 1 (emitted first) ===========
        # HvT[m, i] partial = sum_r sum_u (a_u * mask_r[u,i]) * Wu[r][m,u]
        wuT = [[None] * len(UPT) for _ in range(R)]
        for icp in range(2):  # item column halves (2 x 1000)
            ic0 = icp * 1000
            P = [
                [ps_mm.tile([128, 500], FP, tag="p1", bufs=4, name="P") for _ in range(2)]
                for _ in range(2)
            ]
            for r in range(R):
                for kt, (us, pu) in enumerate(UPT):
                    if icp == 0:
                        wt = res.tile([128, M], BF, tag=f"wuT{r}_{kt}")
                        wuT[r][kt] = wt
                        prep_wT(wu, r, us, pu, wt)
                    msk = scr.tile([128, 1000], BF, tag="mask", bufs=4)
                    nc.vector.tensor_scalar(
                        out=msk[:pu, :], in0=adjb[kt][:pu, ic0 : ic0 + 1000],
                        scalar1=float(r + 1), scalar2=a_fac[kt][:pu, :],
                        op0=ALU.is_equal, op1=ALU.mult,
                    )
                    first = r == 0 and kt == 0
                    last = r == R - 1 and kt == len(UPT) - 1
                    for ic2 in range(2):
                        for mh in range(2):
                            nc.tensor.matmul(
                                P[ic2][mh][:, :],
                                lhsT=wuT[r][kt][:pu, mh * 128 : (mh + 1) * 128],
                                rhs=msk[:pu, ic2 * 500 : ic2 * 500 + 500],
                                start=first, stop=last,
                            )
            for ic2 in range(2):
                for mh in range(2):
                    ev = scr.tile([128, 500], FP, tag="ev", bufs=3)
                    nc.scalar.copy(out=ev[:, :], in_=P[ic2][mh][:, :])
                    nc.sync.dma_start(
                        out=dram_hvT[
                            mh * 128 : (mh + 1) * 128,
                            ic0 + ic2 * 500 : ic0 + ic2 * 500 + 500,
                        ],
                        in_=ev[:, :],
                    )
        nc.gpsimd.collective_compute(
            "AllReduce", ALU.add, replica_groups=QUAD_GROUPS,
            ins=[dram_hvT.opt()], outs=[dram_hvT_red.opt()],
        )

        # =========== USER-side pass 1 ===========
        # HuT[m, u] partial = sum_r sum_i (b_i * maskT_r[i,u]) * Wi[r][m,i]
        P = [
            [ps_mm.tile([128, 500], FP, tag="p1", bufs=4, name="P") for _ in range(2)]
            for _ in range(2)
        ]
        for r in range(R):
            for kt, (isrt, pi) in enumerate(IPT):
                wt = scr.tile([128, M], BF, tag="wiT", bufs=3)
                prep_wT(wi, r, isrt, pi, wt)
                msk = scr.tile([128, 1000], BF, tag="mask", bufs=4)
                nc.vector.tensor_scalar(
                    out=msk[:pi, :], in0=adjT[kt][:pi, :],
                    scalar1=float(r + 1), scalar2=b_fac[kt][:pi, :],
                    op0=ALU.is_equal, op1=ALU.mult,
                )
                first = r == 0 and kt == 0
                last = r == R - 1 and kt == len(IPT) - 1
                for uc in range(2):
                    for mh in range(2):
                        nc.tensor.matmul(
                            P[uc][mh][:, :],
                            lhsT=wt[:pi, mh * 128 : (mh + 1) * 128],
                            rhs=msk[:pi, uc * 500 : uc * 500 + 500],
                            start=first, stop=last,
                        )
        for uc in range(2):
            for mh in range(2):
                ev = scr.tile([128, 500], FP, tag="ev", bufs=3)
                nc.scalar.copy(out=ev[:, :], in_=P[uc][mh][:, :])
                nc.sync.dma_start(
                    out=dram_huT[mh * 128 : (mh + 1) * 128, uc * 500 : uc * 500 + 500],
                    in_=ev[:, :],
                )
        nc.gpsimd.collective_compute(
            "AllReduce", ALU.add, replica_groups=PAIR_GROUPS,
            ins=[dram_huT.opt()], outs=[dram_huT_red.opt()],
        )

        # =========== Pass 2 shared small weights ===========
        def load_t_small(w_dram, rows, cols, nm):
            # returns bf16 transposed [cols, rows] (cols<=128, rows<=128)
            f = scr.tile([128, 256], FP, tag="smf")
            nc.sync.dma_start(out=f[:rows, :cols], in_=w_dram[:, :])
            bmat = scr.tile([128, 256], BF, tag="smb")
            nc.scalar.copy(out=bmat[:rows, :cols], in_=f[:rows, :cols])
            pt_ps = ps_tr.tile([128, 256], BF, tag="trp")
            nc.tensor.transpose(
                pt_ps[:cols, :rows], bmat[:rows, :cols], ident[:rows, :rows]
            )
            outt = res.tile([128, 128], BF, tag=f"smT{nm}")
            nc.scalar.copy(out=outt[:cols, :rows], in_=pt_ps[:cols, :rows])
            return outt

        dwT = []  # dense_W^T as two [128, OUT] tiles
        for mh in range(2):
            f = scr.tile([128, 256], FP, tag="smf")
            nc.sync.dma_start(out=f[:OUT, :128], in_=dw[:, mh * 128 : (mh + 1) * 128])
            bmat = scr.tile([128, 256], BF, tag="smb")
            nc.scalar.copy(out=bmat[:OUT, :128], in_=f[:OUT, :128])
            pt_ps = ps_tr.tile([128, 256], BF, tag="trp")
            nc.tensor.transpose(pt_ps[:128, :OUT], bmat[:OUT, :128], ident[:OUT, :OUT])
            t = res.tile([128, OUT], BF, tag=f"dwT{mh}")
            nc.scalar.copy(out=t[:, :], in_=pt_ps[:128, :OUT])
            dwT.append(t)

        uw1T = load_t_small(uw1, SIDE, FDIM, "uw1")  # [FDIM, SIDE]
        uw2T = load_t_small(uw2, OUT, SIDE, "uw2")  # [SIDE, OUT]
        vw1T = load_t_small(vw1, SIDE, FDIM, "vw1")
        vw2T = load_t_small(vw2, OUT, SIDE, "vw2")
        ub1_t = res.tile([SIDE, 1], FP, tag="biasu")
        nc.sync.dma_start(out=ub1_t[:, :], in_=ub1[:, :])
        vb1_t = res.tile([SIDE, 1], FP, tag="biasv")
        nc.sync.dma_start(out=vb1_t[:, :], in_=vb1[:, :])

        # release pass-1 PSUM pools, open pass-2 pools
        ps_mm.release()
        ps_cs.release()
        ps_p2 = ctx.enter_context(tc.tile_pool(name="ps_p2", bufs=2, space="PSUM"))

        def pass2(h_red_dram, side_dram, w1T, bias_t, w2T, fac, tiles, n, o_dram, nm):
            # side features -> F^T = relu(w1 @ sf^T + b)  [SIDE, n] bf16
            sfT = res.tile([128, n], BF, tag=f"sfT{nm}")
            for t, (s, p) in enumerate(tiles):
                f = scr.tile([128, FDIM], FP, tag="p2f")
                nc.sync.dma_start(out=f[:p, :], in_=side_dram[s : s + p, :])
                bmat = scr.tile([128, FDIM], BF, tag="p2b")
                nc.scalar.copy(out=bmat[:p, :], in_=f[:p, :])
                pt_ps = ps_tr.tile([128, 256], BF, tag="trp")
                nc.tensor.transpose(pt_ps[:FDIM, :p], bmat[:p, :], ident[:p, :p])
                nc.scalar.copy(out=sfT[:FDIM, s : s + p], in_=pt_ps[:FDIM, :p])
            fT = res.tile([SIDE, n], BF, tag=f"fT{nm}")
            for c in range(0, n, 500):
                pf = ps_p2.tile([SIDE, 500], FP, tag="pf")
                nc.tensor.matmul(
                    pf[:, :], lhsT=w1T[:FDIM, :SIDE], rhs=sfT[:FDIM, c : c + 500],
                    start=True, stop=True,
                )
                nc.scalar.activation(
                    out=fT[:, c : c + 500], in_=pf[:, :], func=AF.Relu,
                    bias=bias_t[:, :],
                )
            # H^T: relu + bf16
            hT = []
            for mh in range(2):
                hb = res.tile([128, n], BF, tag=f"hT{nm}{mh}")
                for c in range(0, n, 1000):
                    hf = scr.tile([128, 1000], FP, tag="p2h")
                    nc.sync.dma_start(
                        out=hf[:, :],
                        in_=h_red_dram[mh * 128 : (mh + 1) * 128, c : c + 1000],
                    )
                    nc.scalar.activation(
                        out=hb[:, c : c + 1000], in_=hf[:, :], func=AF.Relu
                    )
                hT.append(hb)
            # output rows
            for t, (s, p) in enumerate(tiles):
                pa = ps_p2.tile([128, OUT], FP, tag="pa")
                for mh in range(2):
                    nc.tensor.matmul(
                        pa[:p, :], lhsT=hT[mh][:, s : s + p], rhs=dwT[mh][:, :],
                        start=(mh == 0), stop=(mh == 1),
                    )
                sa = scr.tile([128, OUT], FP, tag="p2sa")
                nc.scalar.activation(
                    out=sa[:p, :], in_=pa[:p, :], func=AF.Copy, scale=fac[t][:p, :]
                )
                pb = ps_p2.tile([128, OUT], FP, tag="pb")
                nc.tensor.matmul(
                    pb[:p, :], lhsT=fT[:SIDE, s : s + p], rhs=w2T[:SIDE, :OUT],
                    start=True, stop=True,
                )
                so = scr.tile([128, OUT], FP, tag="p2so")
                nc.vector.tensor_tensor(
                    out=so[:p, :], in0=pb[:p, :], in1=sa[:p, :], op=ALU.add
                )
                ro = scr.tile([128, OUT], FP, tag="p2ro")
                nc.scalar.activation(out=ro[:p, :], in_=so[:p, :], func=AF.Relu)
                nc.sync.dma_start(out=o_dram[s : s + p, :], in_=ro[:p, :])

        pass2(dram_hvT_red, vf, vw1T, vb1_t, vw2T, b_fac, IPT, BI, v_out, "v")
        pass2(dram_huT_red, uf, uw1T, ub1_t, uw2T, a_fac, UPT, BU, u_out, "u")

    nc.compile()
    return nc


_CACHE = {}


def _get_program():
    if "nc" not in _CACHE:
        _CACHE["nc"] = build_program()
    return _CACHE["nc"]


def make_in_maps(inputs):
    adj = np.asarray(inputs["adj_matrix"], dtype=np.int32)
    u_sf = np.asarray(inputs["u_sideFeat"], dtype=np.float32)
    v_sf = np.asarray(inputs["v_sideFeat"], dtype=np.float32)
    msg_W = np.asarray(inputs["msg_W"], dtype=np.float32)
    dense_W = np.asarray(inputs["dense_W"], dtype=np.float32)
    u_W1 = np.asarray(inputs["u_W1"], dtype=np.float32)
    u_b1 = np.asarray(inputs["u_b1"], dtype=np.float32).reshape(SIDE, 1)
    u_W2 = np.asarray(inputs["u_W2"], dtype=np.float32)
    v_W1 = np.asarray(inputs["v_W1"], dtype=np.float32)
    v_b1 = np.asarray(inputs["v_b1"], dtype=np.float32).reshape(SIDE, 1)
    v_W2 = np.asarray(inputs["v_W2"], dtype=np.float32)

    in_maps = []
    for a in range(GA):
        for b in range(GB):
            in_maps.append(
                {
                    "adj_blk": np.ascontiguousarray(
                        adj[a * BU : (a + 1) * BU, b * BI : (b + 1) * BI]
                    ),
                    "wi": np.ascontiguousarray(
                        msg_W[:, :, NU + b * BI : NU + (b + 1) * BI]
                    ),
                    "wu": np.ascontiguousarray(msg_W[:, :, a * BU : (a + 1) * BU]),
                    "uf": np.ascontiguousarray(u_sf[a * BU : (a + 1) * BU]),
                    "vf": np.ascontiguousarray(v_sf[b * BI : (b + 1) * BI]),
                    "dw": dense_W,
                    "uw1": u_W1,
                    "ub1": u_b1,
                    "uw2": u_W2,
                    "vw1": v_W1,
                    "vb1": v_b1,
                    "vw2": v_W2,
                }
            )
    return in_maps


def assemble(results):
    U = np.empty((NU, OUT), np.float32)
    V = np.empty((NI, OUT), np.float32)
    for a in range(GA):
        U[a * BU : (a + 1) * BU] = results[a * GB]["u_out"]
    for b in range(GB):
        V[b * BI : (b + 1) * BI] = results[b]["v_out"]
    return (U, V)


def kernel(**inputs):
    from concourse.bass_utils import run_bass_kernel_spmd

    nc = _get_program()
    res = run_bass_kernel_spmd(nc, make_in_maps(inputs), core_ids=list(range(NCORES)))
    return assemble(res.results)
